# revision 1
# baseline (speedup 1.0000x reference)
"""CrossAttention kernel for Trainium2 (8 NeuronCores).

Problem: B=4, Sq=Sk=2048, H=16 heads, D=64, NUM_HIDDEN=1024.
query/key are (B, S, 1) and Wq/Wk are (1, 1024) -- the q/k projections are
rank-1.  The attention logits therefore factor as

  logits[i,j] = (q_i . k_j)/8 = A_h x_i y_j/8 + C_h x_i/8 + E_h y_j/8 + F_h/8

with per-head scalars A_h = Wq_h.Wk_h, E_h = Wk_h.bq_h (x = query[...,0],
y = key[...,0]).  Terms constant in j cancel under softmax over j, so

  attn[i, :] = softmax_j( scale_j * x_i + bias_j ),
  scale_j = A_h y_j / 8,   bias_j = E_h y_j / 8.

On device (per core: one batch b, 8 heads):
  1. V projection: V = value_b @ Wv[:, headcols] + bv   (PE matmul, K=1024)
  2. T[j, i] = exp(scale_j * x_i + bias_j) -- ONE ScalarE activation per
     (head, j-tile): exp with per-partition scale/bias, free dim = 2048.
  3. numerator/denominator in one PE matmul: lhsT = [V_h | 1] (j x 65),
     rhs = T (j x 512 chunks), accumulated over 16 j-tiles in PSUM.
  4. PE-transpose [65, 128] chunks -> [128, 65], reciprocal of row 65,
     tensor_scalar_mul, DMA out.  Softmax normalization (divide by Z)
     happens here; no max-subtraction needed (|logit| < 80 asserted on host).

Sharding: core c -> batch b = c // 2, head group g = c % 2 (8 heads each).
"""

import sys

import numpy as np

for _p in ("/opt/trn_rl_repo",):
    if _p not in sys.path:
        sys.path.insert(0, _p)

B = 4
S = 2048
H = 16
D = 64
NH = 1024
P = 128
JT = S // P          # 16 j-tiles
HPC = 8              # heads per core
HT = NH // P         # 8 hidden tiles
IC = 4               # i-chunks of 512
ICW = 512
N_CORES = 8

# float32r: fp32 bits, PE "replicated" mode -> 1 cyc/row (vs 4 for fp32)
import os as _os

USE_F32R = _os.environ.get("USE_F32R", "1") == "1"

_cache = {}


def _build_program():
    import concourse.bass as bass  # noqa: F401
    import concourse.mybir as mybir
    from concourse import bacc
    from concourse.masks import make_identity
    from concourse.tile import TileContext

    f32 = mybir.dt.float32
    mdt = mybir.dt.float32r if USE_F32R else f32

    nc = bacc.Bacc(trn_type="TRN2")

    valueT = nc.dram_tensor("valueT", [HT, P, S], mdt, kind="ExternalInput")
    wv = nc.dram_tensor("wv", [HT, P, HPC * D], mdt, kind="ExternalInput")
    bvs = nc.dram_tensor("bvs", [1, HPC * D], mdt, kind="ExternalInput")
    # meta: per-partition [sb (JT*HPC) | eb (JT*HPC) | x broadcast (S)]
    meta = nc.dram_tensor("meta", [P, 2 * JT * HPC + S], f32, kind="ExternalInput")
    onesd = nc.dram_tensor("onesd", [P, HPC * JT], mdt, kind="ExternalInput")
    out = nc.dram_tensor("out", [HPC, S, D], f32, kind="ExternalOutput")

    with TileContext(nc) as tc:
        with (
            tc.tile_pool(name="const", bufs=1) as const_pool,
            tc.tile_pool(name="vp", bufs=1) as vp_pool,
            tc.tile_pool(name="vt", bufs=3) as vt_pool,
            tc.tile_pool(name="tt", bufs=3) as t_pool,
            tc.tile_pool(name="ps", bufs=2, space="PSUM") as ps_pool,
            tc.tile_pool(name="av", bufs=4, space="PSUM") as av_pool,
            tc.tile_pool(name="tp", bufs=2, space="PSUM") as tp_pool,
            tc.tile_pool(name="sp", bufs=3) as s_pool,
            tc.tile_pool(name="cp", bufs=3) as c_pool,
            tc.tile_pool(name="op", bufs=3) as o_pool,
        ):
            ident = const_pool.tile([P, P], f32)
            make_identity(nc, ident)
            ones1 = const_pool.tile([1, P], mdt)
            nc.sync.dma_start(ones1[:, :], onesd[0:1, 0:P])
            wv_sb = const_pool.tile([P, HT, HPC * D], mdt)
            nc.sync.dma_start(
                wv_sb[:, :, :], wv[:, :, :].rearrange("ht p d -> p ht d")
            )
            bv_sb = const_pool.tile([1, HPC * D], mdt)
            nc.sync.dma_start(bv_sb[:, :], bvs[:, :])
            meta_sb = const_pool.tile([P, 2 * JT * HPC + S], f32)
            nc.sync.dma_start(meta_sb[:, :], meta[:, :])
            sb_sb = meta_sb[:, 0 : JT * HPC].rearrange(
                "p (jt h) -> p jt h", h=HPC
            )
            eb_sb = meta_sb[:, JT * HPC : 2 * JT * HPC].rearrange(
                "p (jt h) -> p jt h", h=HPC
            )
            xb_sb = meta_sb[:, 2 * JT * HPC : 2 * JT * HPC + S]

            # V-plus: per head, [j-part, jt, D+1]; column D preset to 1.0 so
            # the AV matmul also produces the softmax denominator (row D).
            vp = vp_pool.tile([P, HPC, JT, D + 1], mdt)
            nc.sync.dma_start(
                vp[:, :, :, D : D + 1],
                onesd[:, :].rearrange("p (h jt one) -> p h jt one", h=HPC, one=1),
            )

            # Phase 1: V projection, all 8 heads at once (N = 512 cols)
            for jt in range(JT):
                vt = vt_pool.tile([P, HT, P], mdt)
                nc.sync.dma_start(
                    vt[:, :, :],
                    valueT[:, :, jt * P : (jt + 1) * P].rearrange(
                        "ht p j -> p ht j"
                    ),
                )
                ps = ps_pool.tile([P, HPC * D], f32, space="PSUM")
                for ht in range(HT):
                    nc.tensor.matmul(
                        ps,
                        vt[:, ht, :],
                        wv_sb[:, ht, :],
                        start=(ht == 0),
                        stop=False,
                    )
                nc.tensor.matmul(ps, ones1[:, :], bv_sb[:, :], start=False, stop=True)
                # scatter into per-head V-plus slots (3D strided copy)
                nc.vector.tensor_copy(
                    vp[:, :, jt, 0:D],
                    ps.rearrange("p (h d) -> p h d", h=HPC),
                )

            # Phase 2: attention
            for hl in range(HPC):
                avs = []
                for ic in range(IC):
                    av = av_pool.tile(
                        [D + 1, ICW], f32, name=f"av{ic}", tag="av", space="PSUM"
                    )
                    avs.append(av)
                for jt in range(JT):
                    tte = t_pool.tile([P, S], mdt)
                    nc.scalar.activation(
                        tte,
                        xb_sb,
                        mybir.ActivationFunctionType.Exp,
                        bias=eb_sb[:, jt, hl : hl + 1],
                        scale=sb_sb[:, jt, hl : hl + 1],
                    )
                    for ic in range(IC):
                        nc.tensor.matmul(
                            avs[ic],
                            vp[:, hl, jt, :],
                            tte[:, ic * ICW : (ic + 1) * ICW],
                            start=(jt == 0),
                            stop=(jt == JT - 1),
                        )
                for ic in range(IC):
                    sten = s_pool.tile([D + 1, ICW], f32)
                    nc.vector.tensor_copy(sten, avs[ic])
                    nc.vector.reciprocal(sten[D : D + 1, :], sten[D : D + 1, :])
                    tp = tp_pool.tile([P, IC, D + 1], f32, space="PSUM")
                    for q in range(IC):
                        nc.tensor.transpose(
                            tp[:, q, :],
                            sten[:, q * P : (q + 1) * P],
                            ident[0 : D + 1, 0 : D + 1],
                        )
                    ctile = c_pool.tile([P, IC, D + 1], f32)
                    nc.vector.tensor_copy(ctile, tp)
                    otile = o_pool.tile([P, IC, D], f32)
                    for q in range(IC):
                        nc.vector.tensor_scalar_mul(
                            otile[:, q, :], ctile[:, q, 0:D], ctile[:, q, D : D + 1]
                        )
                    nc.sync.dma_start(
                        out[hl, ic * ICW : (ic + 1) * ICW, :].rearrange(
                            "(q p) d -> p q d", p=P
                        ),
                        otile,
                    )
    nc.compile()  # bacc legalization: wait-splitting, reg alloc, nop fusion
    return nc


def _get_program():
    if "nc" not in _cache:
        _cache["nc"] = _build_program()
    return _cache["nc"]


def kernel(query, key, value, Wq, bq, Wk, bk, Wv, bv):
    from concourse import bass_utils

    query = np.asarray(query, dtype=np.float32)
    key = np.asarray(key, dtype=np.float32)
    value = np.asarray(value, dtype=np.float32)
    Wq = np.asarray(Wq, dtype=np.float32)
    bq = np.asarray(bq, dtype=np.float32)
    Wk = np.asarray(Wk, dtype=np.float32)
    bk = np.asarray(bk, dtype=np.float32)
    Wv = np.asarray(Wv, dtype=np.float32)
    bv = np.asarray(bv, dtype=np.float32)

    wq2 = Wq.reshape(H, D)
    wk2 = Wk.reshape(H, D)
    bq2 = bq.reshape(H, D)
    A = np.einsum("hd,hd->h", wq2, wk2)  # Wq_h . Wk_h
    E = np.einsum("hd,hd->h", wk2, bq2)  # Wk_h . bq_h

    in_maps = []
    for c in range(N_CORES):
        b = c // 2
        g = c % 2
        heads = np.arange(g * HPC, (g + 1) * HPC)
        x = query[b, :, 0]  # (S,)
        y = key[b, :, 0]  # (S,)
        # scale[j, h] = A_h y_j / 8 ; bias[j, h] = E_h y_j / 8
        sc = (y[:, None] * (A[heads] / 8.0)[None, :]).astype(np.float32)
        bi = (y[:, None] * (E[heads] / 8.0)[None, :]).astype(np.float32)
        amax = np.abs(sc * np.abs(x).max() + np.abs(bi)).max()
        assert amax < 80.0, f"logit magnitude {amax} risks fp32 exp overflow"
        in_maps.append(
            {
                "valueT": np.ascontiguousarray(value[b].T).reshape(HT, P, S),
                "wv": np.ascontiguousarray(
                    Wv[:, g * HPC * D : (g + 1) * HPC * D]
                ).reshape(HT, P, HPC * D),
                "bvs": np.ascontiguousarray(
                    bv[g * HPC * D : (g + 1) * HPC * D]
                ).reshape(1, HPC * D),
                "onesd": np.ones((P, HPC * JT), dtype=np.float32),
                "meta": np.concatenate(
                    [
                        sc.reshape(JT, P, HPC).transpose(1, 0, 2).reshape(P, -1),
                        bi.reshape(JT, P, HPC).transpose(1, 0, 2).reshape(P, -1),
                        np.broadcast_to(x, (P, S)),
                    ],
                    axis=1,
                ).astype(np.float32),
            }
        )

    nc = _get_program()
    res = bass_utils.run_bass_kernel_spmd(
        nc, in_maps, core_ids=list(range(N_CORES))
    ).results

    full = np.empty((H * B, S, D), dtype=np.float32)
    for c in range(N_CORES):
        b = c // 2
        g = c % 2
        o = res[c]["out"]
        for hl in range(HPC):
            full[(g * HPC + hl) * B + b] = o[hl]
    return full



# revision 2
# speedup vs baseline: 1.3822x; 1.3822x over previous
"""CrossAttention kernel for Trainium2 (8 NeuronCores).

Problem: B=4, Sq=Sk=2048, H=16 heads, D=64, NUM_HIDDEN=1024.
query/key are (B, S, 1) and Wq/Wk are (1, 1024) -- the q/k projections are
rank-1.  The attention logits therefore factor as

  logits[i,j] = (q_i . k_j)/8 = A_h x_i y_j/8 + C_h x_i/8 + E_h y_j/8 + F_h/8

with per-head scalars A_h = Wq_h.Wk_h, E_h = Wk_h.bq_h (x = query[...,0],
y = key[...,0]).  Terms constant in j cancel under softmax over j, so

  attn[i, :] = softmax_j( scale_j * x_i + bias_j ),
  scale_j = A_h y_j / 8,   bias_j = E_h y_j / 8.

On device (per core: one batch b, 8 heads, everything bf16 on the PE):
  1. V projection: V = value_b @ Wv[:, headcols] + bv   (PE matmul, K=1024)
  2. T[j, i] = exp(scale_j * x_i + bias_j) -- ONE ScalarE activation per
     (head, j-tile): exp with per-partition scale/bias, free dim = 2048,
     bf16 output.  The ScalarE exp stream is the bottleneck engine; a deep
     tte ring buffer keeps it running back-to-back from t=0.
  3. numerator/denominator in one PE matmul: lhsT = [V_h | 1] (j x 65),
     rhs = T (j x 512 chunks), accumulated over 16 j-tiles in PSUM.
  4. PE-transpose [65, 128] chunks -> [128, 65], reciprocal of the Z
     column on [128, 4] per-partition data, tensor_scalar_mul on GpSimd,
     per-head SBUF-accumulated output DMA with 4KB partition lines.

Sharding: core c -> batch b = c // 2, head group g = c % 2 (8 heads each).
"""

import sys

import numpy as np

for _p in ("/opt/trn_rl_repo",):
    if _p not in sys.path:
        sys.path.insert(0, _p)

B = 4
S = 2048
H = 16
D = 64
NH = 1024
P = 128
JT = S // P          # 16 j-tiles
HPC = 8              # heads per core
HT = NH // P         # 8 hidden tiles
IC = 4               # i-chunks of 512
ICW = 512
N_CORES = 8
TBUFS = 20           # tte ring depth: covers ScalarE lead over phase-1 VP

_cache = {}


def _build_program():
    import concourse.bass as bass  # noqa: F401
    import concourse.mybir as mybir
    from concourse import bacc
    from concourse.masks import make_identity
    from concourse.tile import TileContext

    f32 = mybir.dt.float32
    bf16 = mybir.dt.bfloat16

    nc = bacc.Bacc(trn_type="TRN2")

    # [p, ht, j]: partition line = HT*S*2B = 32 KB contiguous
    valueT = nc.dram_tensor("valueT", [P, HT, S], bf16, kind="ExternalInput")
    # [p, ht, col]: partition line = HT*512*2B = 8 KB contiguous
    wv = nc.dram_tensor("wv", [P, HT, HPC * D], bf16, kind="ExternalInput")
    bvs = nc.dram_tensor("bvs", [1, HPC * D], bf16, kind="ExternalInput")
    # meta: per-partition [sb (JT*HPC) | eb (JT*HPC) | x broadcast (S)]
    meta = nc.dram_tensor("meta", [P, 2 * JT * HPC + S], f32, kind="ExternalInput")
    # [hl, p, icq, d]: row i = icq*128 + p; partition line = JT*D*4B = 4 KB
    out = nc.dram_tensor("out", [HPC, P, JT, D], f32, kind="ExternalOutput")

    with TileContext(nc) as tc:
        with (
            tc.tile_pool(name="const", bufs=1) as const_pool,
            tc.tile_pool(name="vp", bufs=1) as vp_pool,
            tc.tile_pool(name="tt", bufs=TBUFS) as t_pool,
            tc.tile_pool(name="ps", bufs=2, space="PSUM") as ps_pool,
            tc.tile_pool(name="av", bufs=4, space="PSUM") as av_pool,
            tc.tile_pool(name="tp", bufs=2, space="PSUM") as tp_pool,
            tc.tile_pool(name="sp", bufs=3) as s_pool,
            tc.tile_pool(name="cp", bufs=3) as c_pool,
            tc.tile_pool(name="rp", bufs=3) as r_pool,
            tc.tile_pool(name="op", bufs=2) as o_pool,
        ):
            ident = const_pool.tile([P, P], f32)
            make_identity(nc, ident)
            ones1 = const_pool.tile([1, P], bf16)
            nc.vector.memset(ones1[:, :], 1.0)
            vt_sb = const_pool.tile([P, HT, S], bf16)
            nc.sync.dma_start(vt_sb[:, :, :], valueT[:, :, :])
            wv_sb = const_pool.tile([P, HT, HPC * D], bf16)
            nc.sync.dma_start(wv_sb[:, :, :], wv[:, :, :])
            bv_sb = const_pool.tile([1, HPC * D], bf16)
            nc.sync.dma_start(bv_sb[:, :], bvs[:, :])
            meta_sb = const_pool.tile([P, 2 * JT * HPC + S], f32)
            nc.sync.dma_start(meta_sb[:, :], meta[:, :])
            sb_sb = meta_sb[:, 0 : JT * HPC].rearrange(
                "p (jt h) -> p jt h", h=HPC
            )
            eb_sb = meta_sb[:, JT * HPC : 2 * JT * HPC].rearrange(
                "p (jt h) -> p jt h", h=HPC
            )
            xb_sb = meta_sb[:, 2 * JT * HPC : 2 * JT * HPC + S]

            # V-plus: per head, [j-part, jt, D+1]; column D preset to 1.0 so
            # the AV matmul also produces the softmax denominator (row D).
            vp = vp_pool.tile([P, HPC, JT, D + 1], bf16)
            nc.gpsimd.memset(vp[:, :, :, D : D + 1], 1.0)

            # Phase 1: V projection, all 8 heads at once (N = 512 cols)
            for jt in range(JT):
                ps = ps_pool.tile([P, HPC * D], f32, space="PSUM")
                for ht in range(HT):
                    nc.tensor.matmul(
                        ps,
                        vt_sb[:, ht, jt * P : (jt + 1) * P],
                        wv_sb[:, ht, :],
                        start=(ht == 0),
                        stop=False,
                    )
                nc.tensor.matmul(ps, ones1[:, :], bv_sb[:, :], start=False, stop=True)
                # scatter into per-head V-plus slots (3D strided copy + cast)
                nc.vector.tensor_copy(
                    vp[:, :, jt, 0:D],
                    ps.rearrange("p (h d) -> p h d", h=HPC),
                )

            # Phase 2: attention
            for hl in range(HPC):
                avs = []
                for ic in range(IC):
                    av = av_pool.tile(
                        [D + 1, ICW], f32, name=f"av{ic}", tag="av", space="PSUM"
                    )
                    avs.append(av)
                for jt in range(JT):
                    tte = t_pool.tile([P, S], bf16)
                    nc.scalar.activation(
                        tte,
                        xb_sb,
                        mybir.ActivationFunctionType.Exp,
                        bias=eb_sb[:, jt, hl : hl + 1],
                        scale=sb_sb[:, jt, hl : hl + 1],
                    )
                    for ic in range(IC):
                        nc.tensor.matmul(
                            avs[ic],
                            vp[:, hl, jt, :],
                            tte[:, ic * ICW : (ic + 1) * ICW],
                            start=(jt == 0),
                            stop=(jt == JT - 1),
                        )
                otile = o_pool.tile([P, IC * IC, D], f32)
                for ic in range(IC):
                    sten = s_pool.tile([D + 1, ICW], f32)
                    nc.vector.tensor_copy(sten, avs[ic])
                    tp = tp_pool.tile([P, IC, D + 1], f32, space="PSUM")
                    for q in range(IC):
                        nc.tensor.transpose(
                            tp[:, q, :],
                            sten[:, q * P : (q + 1) * P],
                            ident[0 : D + 1, 0 : D + 1],
                        )
                    ctile = c_pool.tile([P, IC, D + 1], f32)
                    nc.vector.tensor_copy(ctile, tp)
                    rec = r_pool.tile([P, IC, 1], f32)
                    nc.vector.reciprocal(rec, ctile[:, :, D : D + 1])
                    for q in range(IC):
                        nc.gpsimd.tensor_scalar_mul(
                            otile[:, ic * IC + q, :],
                            ctile[:, q, 0:D],
                            rec[:, q, :],
                        )
                nc.sync.dma_start(out[hl, :, :, :], otile)
    nc.compile()  # bacc legalization: wait-splitting, reg alloc, nop fusion
    return nc


def _get_program():
    if "nc" not in _cache:
        _cache["nc"] = _build_program()
    return _cache["nc"]


def kernel(query, key, value, Wq, bq, Wk, bk, Wv, bv):
    import concourse.mybir as mybir
    from concourse import bass_utils

    bf16 = mybir.dt.np(mybir.dt.bfloat16)

    query = np.asarray(query, dtype=np.float32)
    key = np.asarray(key, dtype=np.float32)
    value = np.asarray(value, dtype=np.float32)
    Wq = np.asarray(Wq, dtype=np.float32)
    bq = np.asarray(bq, dtype=np.float32)
    Wk = np.asarray(Wk, dtype=np.float32)
    bk = np.asarray(bk, dtype=np.float32)
    Wv = np.asarray(Wv, dtype=np.float32)
    bv = np.asarray(bv, dtype=np.float32)

    wq2 = Wq.reshape(H, D)
    wk2 = Wk.reshape(H, D)
    bq2 = bq.reshape(H, D)
    A = np.einsum("hd,hd->h", wq2, wk2)  # Wq_h . Wk_h
    E = np.einsum("hd,hd->h", wk2, bq2)  # Wk_h . bq_h

    in_maps = []
    for c in range(N_CORES):
        b = c // 2
        g = c % 2
        heads = np.arange(g * HPC, (g + 1) * HPC)
        x = query[b, :, 0]  # (S,)
        y = key[b, :, 0]  # (S,)
        # scale[j, h] = A_h y_j / 8 ; bias[j, h] = E_h y_j / 8
        sc = (y[:, None] * (A[heads] / 8.0)[None, :]).astype(np.float32)
        bi = (y[:, None] * (E[heads] / 8.0)[None, :]).astype(np.float32)
        amax = np.abs(sc * np.abs(x).max() + np.abs(bi)).max()
        assert amax < 80.0, f"logit magnitude {amax} risks fp32 exp overflow"
        in_maps.append(
            {
                # [p, ht, j] = value[b][j, ht*128+p]
                "valueT": np.ascontiguousarray(
                    value[b].T.reshape(HT, P, S).transpose(1, 0, 2)
                ).astype(bf16),
                # [p, ht, col] = Wv[ht*128+p, g*512+col]
                "wv": np.ascontiguousarray(
                    Wv[:, g * HPC * D : (g + 1) * HPC * D]
                    .reshape(HT, P, HPC * D)
                    .transpose(1, 0, 2)
                ).astype(bf16),
                "bvs": bv[g * HPC * D : (g + 1) * HPC * D].reshape(1, HPC * D).astype(bf16),
                "meta": np.concatenate(
                    [
                        sc.reshape(JT, P, HPC).transpose(1, 0, 2).reshape(P, -1),
                        bi.reshape(JT, P, HPC).transpose(1, 0, 2).reshape(P, -1),
                        np.broadcast_to(x, (P, S)),
                    ],
                    axis=1,
                ).astype(np.float32),
            }
        )

    nc = _get_program()
    res = bass_utils.run_bass_kernel_spmd(
        nc, in_maps, core_ids=list(range(N_CORES))
    ).results

    full = np.empty((H * B, S, D), dtype=np.float32)
    for c in range(N_CORES):
        b = c // 2
        g = c % 2
        o = res[c]["out"]  # [HPC, P, JT, D]; row i = icq*128 + p
        for hl in range(HPC):
            full[(g * HPC + hl) * B + b] = (
                o[hl].transpose(1, 0, 2).reshape(S, D)
            )
    return full


# revision 15
# speedup vs baseline: 1.4700x; 1.0635x over previous
"""CrossAttention kernel for Trainium2 (8 NeuronCores).

Problem: B=4, Sq=Sk=2048, H=16 heads, D=64, NUM_HIDDEN=1024.
query/key are (B, S, 1) and Wq/Wk are (1, 1024) -- the q/k projections are
rank-1.  The attention logits therefore factor as

  logits[i,j] = (q_i . k_j)/8 = A_h x_i y_j/8 + C_h x_i/8 + E_h y_j/8 + F_h/8

with per-head scalars A_h = Wq_h.Wk_h, E_h = Wk_h.bq_h (x = query[...,0],
y = key[...,0]).  Terms constant in j cancel under softmax over j, so

  attn[i, :] = softmax_j( scale_j * x_i + bias_j ),
  scale_j = A_h y_j / 8,   bias_j = E_h y_j / 8.

On device (per core: one batch b, 8 heads, everything bf16 on the PE):
  1. V projection: V = value_b @ Wv[:, headcols] + bv   (PE matmul, K=1024)
  2. T[j, i] = exp(scale_j * x_i + bias_j) -- ONE ScalarE activation per
     (head, j-tile): exp with per-partition scale/bias, free dim = 2048,
     bf16 output.  The ScalarE exp stream is the bottleneck engine; a deep
     tte ring buffer keeps it running back-to-back from t=0.
  3. numerator/denominator in one PE matmul: lhsT = [V_h | 1] (j x 65),
     rhs = T (j x 512 chunks), accumulated over 16 j-tiles in PSUM.
  4. PE-transpose [65, 128] chunks -> [128, 65], reciprocal of the Z
     column on [128, 4] per-partition data, tensor_scalar_mul on GpSimd,
     per-head SBUF-accumulated output DMA with 4KB partition lines.

Sharding: core c -> batch b = c // 2, head group g = c % 2 (8 heads each).
"""

import sys

import numpy as np

for _p in ("/opt/trn_rl_repo",):
    if _p not in sys.path:
        sys.path.insert(0, _p)

B = 4
S = 2048
H = 16
D = 64
NH = 1024
P = 128
JT = S // P          # 16 j-tiles
HPC = 8              # heads per core
HT = NH // P         # 8 hidden tiles
IC = 4               # i-chunks of 512
ICW = 512
N_CORES = 8
TBUFS = 20           # tte ring depth: covers ScalarE lead over phase-1 VP

_cache = {}


def _build_program():
    import concourse.bass as bass  # noqa: F401
    import concourse.mybir as mybir
    from concourse import bacc
    from concourse.masks import make_identity
    from concourse.tile import TileContext

    f32 = mybir.dt.float32
    bf16 = mybir.dt.bfloat16
    fp16 = mybir.dt.float16

    nc = bacc.Bacc(trn_type="TRN2")

    # [p, ht, j]: partition line = HT*S*2B = 32 KB contiguous
    valueT = nc.dram_tensor("valueT", [P, HT, S], bf16, kind="ExternalInput")
    # [p, ht, col]: partition line = HT*512*2B = 8 KB contiguous
    wv = nc.dram_tensor("wv", [P, HT, HPC * D], bf16, kind="ExternalInput")
    bvs = nc.dram_tensor("bvs", [1, HPC * D], bf16, kind="ExternalInput")
    # meta: per-partition [sb (JT*HPC) | eb (JT*HPC)]
    meta = nc.dram_tensor("meta", [P, 2 * JT * HPC], f32, kind="ExternalInput")
    # x broadcast to all partitions, fp16 (2 elem/cycle on ScalarE)
    xh = nc.dram_tensor("xh", [P, S], f32, kind="ExternalInput")
    # [hl, p, icq, d]: row i = icq*128 + p; partition line = JT*D*4B = 4 KB
    out = nc.dram_tensor("out", [HPC, P, JT, D], f32, kind="ExternalOutput")

    with TileContext(nc) as tc:
        with (
            tc.tile_pool(name="const", bufs=1) as const_pool,
            tc.tile_pool(name="vp", bufs=1) as vp_pool,
            tc.tile_pool(name="tt", bufs=TBUFS) as t_pool,
            tc.tile_pool(name="ps", bufs=2, space="PSUM") as ps_pool,
            tc.tile_pool(name="av", bufs=4, space="PSUM") as av_pool,
            tc.tile_pool(name="tp", bufs=2, space="PSUM") as tp_pool,
            tc.tile_pool(name="sp", bufs=4) as s_pool,
            tc.tile_pool(name="cp", bufs=3) as c_pool,
            tc.tile_pool(name="rp", bufs=3) as r_pool,
            tc.tile_pool(name="op", bufs=2) as o_pool,
        ):
            ident = const_pool.tile([P, P], f32)
            make_identity(nc, ident)
            ones1 = const_pool.tile([1, P], bf16)
            nc.vector.memset(ones1[:, :], 1.0)
            # meta first: the ScalarE exp stream (the bottleneck engine)
            # starts as soon as it lands; valueT is only needed ~15us in.
            meta_sb = const_pool.tile([P, 2 * JT * HPC], f32)
            nc.sync.dma_start(meta_sb[:, :], meta[:, :])
            xh_sb = const_pool.tile([P, S], f32)
            nc.sync.dma_start(xh_sb[:, :], xh[:, :])
            bv_sb = const_pool.tile([1, HPC * D], bf16)
            nc.sync.dma_start(bv_sb[:, :], bvs[:, :])
            wv_sb = const_pool.tile([P, HT, HPC * D], bf16)
            nc.sync.dma_start(wv_sb[:, :, :], wv[:, :, :])
            vt_sb = const_pool.tile([P, HT, S], bf16)
            nc.sync.dma_start(vt_sb[:, :, :], valueT[:, :, :])
            sb_sb = meta_sb[:, 0 : JT * HPC].rearrange(
                "p (jt h) -> p jt h", h=HPC
            )
            eb_sb = meta_sb[:, JT * HPC : 2 * JT * HPC].rearrange(
                "p (jt h) -> p jt h", h=HPC
            )

            # V-plus: per head, [j-part, jt, D+1]; column D preset to 1.0 so
            # the AV matmul also produces the softmax denominator (row D).
            vp = vp_pool.tile([P, HPC, JT, D + 1], bf16)
            nc.gpsimd.memset(vp[:, :, :, D : D + 1], 1.0)

            # Phase 1: V projection, all 8 heads at once (N = 512 cols)
            for jt in range(JT):
                ps = ps_pool.tile([P, HPC * D], f32, space="PSUM")
                for ht in range(HT):
                    nc.tensor.matmul(
                        ps,
                        vt_sb[:, ht, jt * P : (jt + 1) * P],
                        wv_sb[:, ht, :],
                        start=(ht == 0),
                        stop=False,
                    )
                nc.tensor.matmul(ps, ones1[:, :], bv_sb[:, :], start=False, stop=True)
                # scatter into per-head V-plus slots (3D strided copy + cast)
                nc.vector.tensor_copy(
                    vp[:, :, jt, 0:D],
                    ps.rearrange("p (h d) -> p h d", h=HPC),
                )

            # Phase 2: attention
            for hl in range(HPC):
                avs = []
                for ic in range(IC):
                    av = av_pool.tile(
                        [D + 1, ICW], f32, name=f"av{ic}", tag="av", space="PSUM"
                    )
                    avs.append(av)
                for jt in range(JT):
                    tte = t_pool.tile([P, S], bf16)
                    nc.scalar.activation(
                        tte,
                        xh_sb,
                        mybir.ActivationFunctionType.Exp,
                        bias=eb_sb[:, jt, hl : hl + 1],
                        scale=sb_sb[:, jt, hl : hl + 1],
                    )
                    for ic in range(IC):
                        nc.tensor.matmul(
                            avs[ic],
                            vp[:, hl, jt, :],
                            tte[:, ic * ICW : (ic + 1) * ICW],
                            start=(jt == 0),
                            stop=(jt == JT - 1),
                        )
                otile = o_pool.tile([P, IC * IC, D], f32)
                stens = []
                for ic in range(IC):
                    sten = s_pool.tile([D + 1, ICW], f32)
                    nc.vector.tensor_copy(sten, avs[ic])
                    stens.append(sten)
                for ic in range(IC):
                    sten = stens[ic]
                    tp = tp_pool.tile([P, IC, D + 1], f32, space="PSUM")
                    for q in range(IC):
                        nc.tensor.transpose(
                            tp[:, q, :],
                            sten[:, q * P : (q + 1) * P],
                            ident[0 : D + 1, 0 : D + 1],
                        )
                    ctile = c_pool.tile([P, IC, D + 1], f32)
                    nc.vector.tensor_copy(ctile, tp)
                    rec = r_pool.tile([P, IC, 1], f32)
                    nc.vector.reciprocal(rec, ctile[:, :, D : D + 1])
                    for q in range(IC):
                        nc.vector.tensor_scalar_mul(
                            otile[:, ic * IC + q, :],
                            ctile[:, q, 0:D],
                            rec[:, q, :],
                        )
                    nc.sync.dma_start(
                        out[hl, :, ic * IC : (ic + 1) * IC, :],
                        otile[:, ic * IC : (ic + 1) * IC, :],
                    )
    nc.compile()  # bacc legalization: wait-splitting, reg alloc, nop fusion
    return nc


def _get_program():
    if "nc" not in _cache:
        _cache["nc"] = _build_program()
    return _cache["nc"]


def kernel(query, key, value, Wq, bq, Wk, bk, Wv, bv):
    import concourse.mybir as mybir
    from concourse import bass_utils

    bf16 = mybir.dt.np(mybir.dt.bfloat16)

    query = np.asarray(query, dtype=np.float32)
    key = np.asarray(key, dtype=np.float32)
    value = np.asarray(value, dtype=np.float32)
    Wq = np.asarray(Wq, dtype=np.float32)
    bq = np.asarray(bq, dtype=np.float32)
    Wk = np.asarray(Wk, dtype=np.float32)
    bk = np.asarray(bk, dtype=np.float32)
    Wv = np.asarray(Wv, dtype=np.float32)
    bv = np.asarray(bv, dtype=np.float32)

    wq2 = Wq.reshape(H, D)
    wk2 = Wk.reshape(H, D)
    bq2 = bq.reshape(H, D)
    A = np.einsum("hd,hd->h", wq2, wk2)  # Wq_h . Wk_h
    E = np.einsum("hd,hd->h", wk2, bq2)  # Wk_h . bq_h

    in_maps = []
    for c in range(N_CORES):
        b = c // 2
        g = c % 2
        heads = np.arange(g * HPC, (g + 1) * HPC)
        x = query[b, :, 0]  # (S,)
        y = key[b, :, 0]  # (S,)
        # scale[j, h] = A_h y_j / 8 ; bias[j, h] = E_h y_j / 8
        sc = (y[:, None] * (A[heads] / 8.0)[None, :]).astype(np.float32)
        bi = (y[:, None] * (E[heads] / 8.0)[None, :]).astype(np.float32)
        amax = np.abs(sc * np.abs(x).max() + np.abs(bi)).max()
        assert amax < 80.0, f"logit magnitude {amax} risks fp32 exp overflow"
        in_maps.append(
            {
                # [p, ht, j] = value[b][j, ht*128+p]
                "valueT": np.ascontiguousarray(
                    value[b].T.reshape(HT, P, S).transpose(1, 0, 2)
                ).astype(bf16),
                # [p, ht, col] = Wv[ht*128+p, g*512+col]
                "wv": np.ascontiguousarray(
                    Wv[:, g * HPC * D : (g + 1) * HPC * D]
                    .reshape(HT, P, HPC * D)
                    .transpose(1, 0, 2)
                ).astype(bf16),
                "bvs": bv[g * HPC * D : (g + 1) * HPC * D].reshape(1, HPC * D).astype(bf16),
                "meta": np.concatenate(
                    [
                        sc.reshape(JT, P, HPC).transpose(1, 0, 2).reshape(P, -1),
                        bi.reshape(JT, P, HPC).transpose(1, 0, 2).reshape(P, -1),
                    ],
                    axis=1,
                ).astype(np.float32),
                "xh": np.ascontiguousarray(
                    np.broadcast_to(x, (P, S))
                ).astype(np.float32),
            }
        )

    nc = _get_program()
    res = bass_utils.run_bass_kernel_spmd(
        nc, in_maps, core_ids=list(range(N_CORES))
    ).results

    full = np.empty((H * B, S, D), dtype=np.float32)
    for c in range(N_CORES):
        b = c // 2
        g = c % 2
        o = res[c]["out"]  # [HPC, P, JT, D]; row i = icq*128 + p
        for hl in range(HPC):
            full[(g * HPC + hl) * B + b] = (
                o[hl].transpose(1, 0, 2).reshape(S, D)
            )
    return full


# revision 16
# speedup vs baseline: 1.7385x; 1.1827x over previous
"""CrossAttention kernel for Trainium2 (8 NeuronCores).

Problem: B=4, Sq=Sk=2048, H=16 heads, D=64, NUM_HIDDEN=1024.
query/key are (B, S, 1) and Wq/Wk are (1, 1024) -- the q/k projections are
rank-1.  The attention logits therefore factor as

  logits[i,j] = (q_i . k_j)/8 = A_h x_i y_j/8 + C_h x_i/8 + E_h y_j/8 + F_h/8

with per-head scalars A_h = Wq_h.Wk_h, E_h = Wk_h.bq_h (x = query[...,0],
y = key[...,0]).  Terms constant in j cancel under softmax over j, so

  attn[i, :] = softmax_j( scale_j * x_i + bias_j ),
  scale_j = A_h y_j / 8,   bias_j = E_h y_j / 8.

On device (per core: one batch b, 8 heads, everything bf16 on the PE):
  1. V projection: V = value_b @ Wv[:, headcols] + bv   (PE matmul, K=1024)
  2. T[j, i] = exp(scale_j * x_i + bias_j) -- ONE ScalarE activation per
     (head, j-tile): exp with per-partition scale/bias, free dim = 2048,
     bf16 output.  The ScalarE exp stream is the bottleneck engine; a deep
     tte ring buffer keeps it running back-to-back from t=0.
  3. numerator/denominator in one PE matmul: lhsT = [V_h | 1] (j x 65),
     rhs = T (j x 512 chunks), accumulated over 16 j-tiles in PSUM.
  4. PE-transpose [65, 128] chunks -> [128, 65], reciprocal of the Z
     column on [128, 4] per-partition data, tensor_scalar_mul on GpSimd,
     per-head SBUF-accumulated output DMA with 4KB partition lines.

Sharding: core c -> batch b = c // 2, head group g = c % 2 (8 heads each).
"""

import sys

import numpy as np

for _p in ("/opt/trn_rl_repo",):
    if _p not in sys.path:
        sys.path.insert(0, _p)

B = 4
S = 2048
H = 16
D = 64
NH = 1024
P = 128
JT = S // P          # 16 j-tiles
HPC = 8              # heads per core
HT = NH // P         # 8 hidden tiles
IC = 4               # i-chunks of 512
ICW = 512
N_CORES = 8
TBUFS = 16           # tte ring depth: covers ScalarE lead over phase-1 VP

# --- DVE/Pool exp offload (Schraudolph + quadratic mantissa fix) ---
# Quadratic lsq fit of 2^(m-1)/m on [1,2): c(m) = A2 m^2 + A1 m + A0.
# The global 1/A0 factor is folded into every tile (softmax-invariant):
# ScalarE tiles get bias - ln(A0); DVE tiles produce exp/A0 natively.
EXPL = float(2 ** 23)
LOG2E = 1.4426950408889634
A2F, A1F, A0F = 0.22574, -0.66667151, 1.43449126
B2F = A2F / A0F
B1F = A1F / A0F
import numpy as _np_for_mask
MASKF = float(_np_for_mask.int32(0x007FFFFF).view(_np_for_mask.float32))
# per-head offload plan: jt -> engine for the affine+round step
OFF_JTS = {2: "pool", 5: "pool", 8: "pool", 11: "pool", 14: "dve"}

_cache = {}


def _register_exp_op():
    from concourse import dve_ops as dmod
    from concourse.dve_spec import (
        C0,
        C1,
        C2,
        AluOp,
        Bin,
        One,
        Spec,
        Src0,
        Src1,
        lower,
    )
    from concourse.dve_spec import _has_src1 as has_src1
    from concourse.dve_uop import DveOpSpec

    name = "EXP_SCHRAUD_ANT"
    for o in dmod.OPS:
        if o.name == name:
            return o
    m = Bin(AluOp.BITWISE_OR, Bin(AluOp.BITWISE_AND, Src0, C0), C1)
    body = ((m * C2 + Src1) * m + One) * Src0

    def _ref(in0, in1, s0, s1, imm2):
        bits = np.asarray(in0, np.float32).view(np.int32)
        mk = np.float32(s0).view(np.int32)
        eb = np.float32(s1).view(np.int32)
        mm = ((bits & mk) | eb).view(np.float32)
        return ((mm * np.float32(imm2) + in1) * mm + np.float32(1.0)) * np.asarray(
            in0, np.float32
        )

    spec = Spec(body=body, reference=_ref)
    row = dmod._CUSTOM_DVE_ROW_BASE + len(dmod.OPS)
    shas = {}
    for ver in ("v3", "v4"):
        u = lower(spec, ver=ver)
        shas[ver] = DveOpSpec(
            name=name, opcode=row, uops=u, rd1_en=has_src1(spec)
        ).sha(ver)
    op = dmod.DveOp(name, spec, subdim=False, uops_sha=shas)
    dmod.OPS.append(op)
    dmod.CUSTOM_DVE_SPECS[name] = spec
    dmod._SUB_OPCODE_FOR_NAME[name] = row
    return op


def _build_program():
    import concourse.bass as bass  # noqa: F401
    import concourse.mybir as mybir
    from concourse import bacc
    from concourse.masks import make_identity
    from concourse.tile import TileContext

    f32 = mybir.dt.float32
    bf16 = mybir.dt.bfloat16
    i32 = mybir.dt.int32

    expop = _register_exp_op()

    nc = bacc.Bacc(trn_type="TRN2")

    # [p, ht, j]: partition line = HT*S*2B = 32 KB contiguous
    valueT = nc.dram_tensor("valueT", [P, HT, S], bf16, kind="ExternalInput")
    # [p, ht, col]: partition line = HT*512*2B = 8 KB contiguous
    wv = nc.dram_tensor("wv", [P, HT, HPC * D], bf16, kind="ExternalInput")
    bvs = nc.dram_tensor("bvs", [1, HPC * D], bf16, kind="ExternalInput")
    # meta: per-partition [sb | eb-ln(A0) | ss | bb], each JT*HPC wide
    meta = nc.dram_tensor("meta", [P, 4 * JT * HPC], f32, kind="ExternalInput")
    # x broadcast to all partitions, fp16 (2 elem/cycle on ScalarE)
    xh = nc.dram_tensor("xh", [P, S], f32, kind="ExternalInput")
    # [hl, p, icq, d]: row i = icq*128 + p; partition line = JT*D*4B = 4 KB
    out = nc.dram_tensor("out", [HPC, P, JT, D], f32, kind="ExternalOutput")

    with TileContext(nc) as tc:
        with (
            tc.tile_pool(name="const", bufs=1) as const_pool,
            tc.tile_pool(name="vp", bufs=1) as vp_pool,
            tc.tile_pool(name="tt", bufs=TBUFS) as t_pool,
            tc.tile_pool(name="zz", bufs=3) as z_pool,
            tc.tile_pool(name="ps", bufs=2, space="PSUM") as ps_pool,
            tc.tile_pool(name="av", bufs=4, space="PSUM") as av_pool,
            tc.tile_pool(name="tp", bufs=2, space="PSUM") as tp_pool,
            tc.tile_pool(name="sp", bufs=4) as s_pool,
            tc.tile_pool(name="cp", bufs=3) as c_pool,
            tc.tile_pool(name="rp", bufs=3) as r_pool,
            tc.tile_pool(name="op", bufs=2) as o_pool,
        ):
            ident = const_pool.tile([P, P], f32)
            make_identity(nc, ident)
            ones1 = const_pool.tile([1, P], bf16)
            nc.vector.memset(ones1[:, :], 1.0)
            # meta first: the ScalarE exp stream (the bottleneck engine)
            # starts as soon as it lands; valueT is only needed ~15us in.
            meta_sb = const_pool.tile([P, 4 * JT * HPC], f32)
            nc.sync.dma_start(meta_sb[:, :], meta[:, :])
            xh_sb = const_pool.tile([P, S], f32)
            nc.sync.dma_start(xh_sb[:, :], xh[:, :])
            bv_sb = const_pool.tile([1, HPC * D], bf16)
            nc.sync.dma_start(bv_sb[:, :], bvs[:, :])
            wv_sb = const_pool.tile([P, HT, HPC * D], bf16)
            nc.sync.dma_start(wv_sb[:, :, :], wv[:, :, :])
            vt_sb = const_pool.tile([P, HT, S], bf16)
            nc.sync.dma_start(vt_sb[:, :, :], valueT[:, :, :])
            sb_sb = meta_sb[:, 0 : JT * HPC].rearrange(
                "p (jt h) -> p jt h", h=HPC
            )
            eb_sb = meta_sb[:, JT * HPC : 2 * JT * HPC].rearrange(
                "p (jt h) -> p jt h", h=HPC
            )
            ss_sb = meta_sb[:, 2 * JT * HPC : 3 * JT * HPC].rearrange(
                "p (jt h) -> p jt h", h=HPC
            )
            bb_sb = meta_sb[:, 3 * JT * HPC : 4 * JT * HPC].rearrange(
                "p (jt h) -> p jt h", h=HPC
            )
            b1sb = const_pool.tile([P, S], f32)
            nc.gpsimd.memset(b1sb[:, :], B1F)

            # V-plus: per head, [j-part, jt, D+1]; column D preset to 1.0 so
            # the AV matmul also produces the softmax denominator (row D).
            vp = vp_pool.tile([P, HPC, JT, D + 1], bf16)
            nc.gpsimd.memset(vp[:, :, :, D : D + 1], 1.0)

            # Phase 1: V projection, all 8 heads at once (N = 512 cols)
            for jt in range(JT):
                ps = ps_pool.tile([P, HPC * D], f32, space="PSUM")
                for ht in range(HT):
                    nc.tensor.matmul(
                        ps,
                        vt_sb[:, ht, jt * P : (jt + 1) * P],
                        wv_sb[:, ht, :],
                        start=(ht == 0),
                        stop=False,
                    )
                nc.tensor.matmul(ps, ones1[:, :], bv_sb[:, :], start=False, stop=True)
                # scatter into per-head V-plus slots (3D strided copy + cast)
                nc.vector.tensor_copy(
                    vp[:, :, jt, 0:D],
                    ps.rearrange("p (h d) -> p h d", h=HPC),
                )

            # Phase 2: attention
            for hl in range(HPC):
                avs = []
                for ic in range(IC):
                    av = av_pool.tile(
                        [D + 1, ICW], f32, name=f"av{ic}", tag="av", space="PSUM"
                    )
                    avs.append(av)
                for jt in range(JT):
                    tte = t_pool.tile([P, S], bf16)
                    eng = OFF_JTS.get(jt)
                    if eng is None:
                        nc.scalar.activation(
                            tte,
                            xh_sb,
                            mybir.ActivationFunctionType.Exp,
                            bias=eb_sb[:, jt, hl : hl + 1],
                            scale=sb_sb[:, jt, hl : hl + 1],
                        )
                    else:
                        z = z_pool.tile([P, S], i32)
                        tsc = nc.gpsimd if eng == "pool" else nc.vector
                        tsc.tensor_scalar(
                            z,
                            xh_sb,
                            ss_sb[:, jt, hl : hl + 1],
                            bb_sb[:, jt, hl : hl + 1],
                            mybir.AluOpType.mult,
                            mybir.AluOpType.add,
                        )
                        nc.vector._custom_dve(
                            expop,
                            out=tte,
                            in0=z.bitcast(f32),
                            in1=b1sb,
                            s0=MASKF,
                            s1=1.0,
                            imm2=B2F,
                        )
                    for ic in range(IC):
                        nc.tensor.matmul(
                            avs[ic],
                            vp[:, hl, jt, :],
                            tte[:, ic * ICW : (ic + 1) * ICW],
                            start=(jt == 0),
                            stop=(jt == JT - 1),
                        )
                otile = o_pool.tile([P, IC * IC, D], f32)
                stens = []
                for ic in range(IC):
                    sten = s_pool.tile([D + 1, ICW], f32)
                    nc.vector.tensor_copy(sten, avs[ic])
                    stens.append(sten)
                for ic in range(IC):
                    sten = stens[ic]
                    tp = tp_pool.tile([P, IC, D + 1], f32, space="PSUM")
                    for q in range(IC):
                        nc.tensor.transpose(
                            tp[:, q, :],
                            sten[:, q * P : (q + 1) * P],
                            ident[0 : D + 1, 0 : D + 1],
                        )
                    ctile = c_pool.tile([P, IC, D + 1], f32)
                    nc.vector.tensor_copy(ctile, tp)
                    rec = r_pool.tile([P, IC, 1], f32)
                    nc.vector.reciprocal(rec, ctile[:, :, D : D + 1])
                    for q in range(IC):
                        nc.vector.tensor_scalar_mul(
                            otile[:, ic * IC + q, :],
                            ctile[:, q, 0:D],
                            rec[:, q, :],
                        )
                    nc.sync.dma_start(
                        out[hl, :, ic * IC : (ic + 1) * IC, :],
                        otile[:, ic * IC : (ic + 1) * IC, :],
                    )
    nc.compile()  # bacc legalization: wait-splitting, reg alloc, nop fusion
    return nc


def _get_program():
    if "nc" not in _cache:
        _cache["nc"] = _build_program()
    return _cache["nc"]


def kernel(query, key, value, Wq, bq, Wk, bk, Wv, bv):
    import concourse.mybir as mybir
    from concourse import bass_utils

    bf16 = mybir.dt.np(mybir.dt.bfloat16)

    query = np.asarray(query, dtype=np.float32)
    key = np.asarray(key, dtype=np.float32)
    value = np.asarray(value, dtype=np.float32)
    Wq = np.asarray(Wq, dtype=np.float32)
    bq = np.asarray(bq, dtype=np.float32)
    Wk = np.asarray(Wk, dtype=np.float32)
    bk = np.asarray(bk, dtype=np.float32)
    Wv = np.asarray(Wv, dtype=np.float32)
    bv = np.asarray(bv, dtype=np.float32)

    wq2 = Wq.reshape(H, D)
    wk2 = Wk.reshape(H, D)
    bq2 = bq.reshape(H, D)
    A = np.einsum("hd,hd->h", wq2, wk2)  # Wq_h . Wk_h
    E = np.einsum("hd,hd->h", wk2, bq2)  # Wk_h . bq_h

    in_maps = []
    for c in range(N_CORES):
        b = c // 2
        g = c % 2
        heads = np.arange(g * HPC, (g + 1) * HPC)
        x = query[b, :, 0]  # (S,)
        y = key[b, :, 0]  # (S,)
        # scale[j, h] = A_h y_j / 8 ; bias[j, h] = E_h y_j / 8
        sc = (y[:, None] * (A[heads] / 8.0)[None, :]).astype(np.float32)
        bi = (y[:, None] * (E[heads] / 8.0)[None, :]).astype(np.float32)
        amax = np.abs(sc * np.abs(x).max() + np.abs(bi)).max()
        assert amax < 80.0, f"logit magnitude {amax} risks fp32 exp overflow"
        in_maps.append(
            {
                # [p, ht, j] = value[b][j, ht*128+p]
                "valueT": np.ascontiguousarray(
                    value[b].T.reshape(HT, P, S).transpose(1, 0, 2)
                ).astype(bf16),
                # [p, ht, col] = Wv[ht*128+p, g*512+col]
                "wv": np.ascontiguousarray(
                    Wv[:, g * HPC * D : (g + 1) * HPC * D]
                    .reshape(HT, P, HPC * D)
                    .transpose(1, 0, 2)
                ).astype(bf16),
                "bvs": bv[g * HPC * D : (g + 1) * HPC * D].reshape(1, HPC * D).astype(bf16),
                "meta": np.concatenate(
                    [
                        sc.reshape(JT, P, HPC).transpose(1, 0, 2).reshape(P, -1),
                        (bi - np.log(A0F))
                        .reshape(JT, P, HPC)
                        .transpose(1, 0, 2)
                        .reshape(P, -1),
                        (sc * LOG2E * EXPL)
                        .reshape(JT, P, HPC)
                        .transpose(1, 0, 2)
                        .reshape(P, -1),
                        (bi * LOG2E * EXPL + 127.0 * EXPL)
                        .reshape(JT, P, HPC)
                        .transpose(1, 0, 2)
                        .reshape(P, -1),
                    ],
                    axis=1,
                ).astype(np.float32),
                "xh": np.ascontiguousarray(
                    np.broadcast_to(x, (P, S))
                ).astype(np.float32),
            }
        )

    nc = _get_program()
    res = bass_utils.run_bass_kernel_spmd(
        nc, in_maps, core_ids=list(range(N_CORES))
    ).results

    full = np.empty((H * B, S, D), dtype=np.float32)
    for c in range(N_CORES):
        b = c // 2
        g = c % 2
        o = res[c]["out"]  # [HPC, P, JT, D]; row i = icq*128 + p
        for hl in range(HPC):
            full[(g * HPC + hl) * B + b] = (
                o[hl].transpose(1, 0, 2).reshape(S, D)
            )
    return full
